# revision 47
# baseline (speedup 1.0000x reference)
"""Trainium2 Bass kernel for GQA attention block (nn_Attention_20272245637793).

Reference computation (B=2, S=2048, H=2048, 16 q heads / 8 kv heads, D=128):
    q = hs @ Wq.T ; k = hs @ Wk.T ; v = hs @ Wv.T
    rope(q), rope(k); causal softmax(q k^T / sqrt(D)) @ v ; out @ Wo.T

Sharding (8 cores): core i = (b, g) with b = i // 4 (data-parallel over
batch), g = i % 4 (tensor-parallel over kv-head groups; kv heads {2g, 2g+1},
q heads {4g..4g+3}).  Each core computes 1/8 of every GEMM and a partial
o_proj over its 512 head-dims; the host sums the 4 partials per batch
(cheap, off-device) instead of an on-device all-reduce.

Per-core dataflow (matmul operands fp16, PSUM accumulation fp32):
  warm-up: ~18 throwaway matmuls on a zeroed tile keep the PE busy across
           the HAM 3.4us activity window while the first DMAs land, so
           real matmuls start at the full (unthrottled) clock.
  phase 1: QK^T projections produce q^T/k^T in [d_head(part) x S(free)]
           layout directly (weights stationary, hs^T moving); RoPE applied
           on the PSUM->SBUF path with 4 DVE ops per tile using
           host-precomputed cos / (+/-)sin tables.  The first hs block
           runs its contraction in two half passes (chunks 0..7 then
           8..15) so the PE starts before the full weights have landed.
           V is computed NON-transposed ([S x d]) with hs^T slices as the
           stationary operand, ones-column appended (denominator trick),
           drained by the otherwise-idle ScalarE.
  phase 2: per q head: scores^T tiles = K^T-chunk (stationary) @ q^T
           (moving) -> PSUM [k_pos(part) x q(free)] in 1536-wide 3-bank
           tiles (3 matmuls fill, ONE wide exp drains via ScalarE with
           scale=1/sqrt(D) fused); causal handled by skipping fully-masked
           tiles + one 0/1 mask multiply on diagonal tiles.  PV matmuls
           (exp'd score tiles stationary against V'; output column 128 is
           the softmax denominator) interleave into the scores loop with a
           4-stripe lag, filling the PE bubbles left by the pacing exp
           stream.  Normalize = DVE reciprocal + per-partition scalar
           multiply into an SBUF staging tile; the [q x d] -> [d x q]
           transpose for o_proj runs OFF the PE as a DMA XBAR transpose
           on the idle sync ring.
  phase 3: o_proj partial out^T[h, s] = Wo-slice^T (stationary) @ attn^T
           (moving), mt-outer so each 128-row strip is stored with ONE
           wide fp16 DMA; psum drains alternate DVE/ScalarE; host sums/
           transposes partials in fp32.

DMA layout: all input tensors are shaped so per-partition lines are
2-16 KiB contiguous, split and ring-ordered by consumption order --
descriptor count and trigger serialization gate the kernel lead-in,
not bandwidth.

Built on bacc.Bacc (not raw bass.Bass): TRN2 instructions can carry at most
ONE semaphore wait; Bacc.compile() legalizes multi-wait instructions via
move_matmul_waits_to_ldweights + generate_event_semaphores.
"""

import sys

sys.path.insert(0, "/opt/trn_rl_repo")

import numpy as np
from contextlib import ExitStack

B = 2
S = 2048
H = 2048
D = 128
NQ = 4          # q heads per core
NKVL = 2        # kv heads per core
HC = H // 128   # 16 h-chunks (contraction)
NB = 8          # hs^T column blocks of 256 for projections
BW = S // NB    # 256
ST = S // 128   # 16 s-tiles / k-chunks / q-tiles
SCALE = 1.0 / np.sqrt(D)

# stripe c of the exp'd transposed scores covers q in [128c, S); offsets of
# the stripes packed into one [128, sum] sbuf tile
STRIPE_LEN = [S - 128 * c for c in range(ST)]
STRIPE_OFF = np.concatenate([[0], np.cumsum(STRIPE_LEN)]).tolist()
PT_TOTAL = STRIPE_OFF[-1]  # 17408

# dtype for matmul operands (PSUM accumulation is always fp32).  fp32 runs
# every matmul as a hi/lo double pass on the PE; float16 is single-pass
# (2x) with ~1e-3 accuracy, and halves DMA/SBUF for those tensors.
MM_DT = "float16"

_CACHE = {}


def _build_program():
    import concourse.tile as tile
    from concourse import bacc, mybir

    f32 = mybir.dt.float32
    fmm = getattr(mybir.dt, MM_DT)
    nc = bacc.Bacc()

    hsT_d = nc.declare_dram_parameter("hsT", [NB, 128, HC, BW], fmm, isOutput=False)
    wq_d = nc.declare_dram_parameter("wq", [128, HC, 128 * NQ], fmm, isOutput=False)
    wk_d = nc.declare_dram_parameter("wk", [128, HC, 128 * NKVL], fmm, isOutput=False)
    wv_d = nc.declare_dram_parameter("wv", [128, HC, 128 * NKVL], fmm, isOutput=False)
    wo_d = nc.declare_dram_parameter("wo", [128, NQ, H], fmm, isOutput=False)
    cos_d = nc.declare_dram_parameter("cosf", [128, S], fmm, isOutput=False)
    sin_d = nc.declare_dram_parameter("sins", [128, S], fmm, isOutput=False)
    mask_d = nc.declare_dram_parameter("mask", [128, 128], fmm, isOutput=False)
    outT_d = nc.declare_dram_parameter("outT", [H, S], fmm, isOutput=True)

    with tile.TileContext(nc) as tc, ExitStack() as top:
        # tiles that live across phases
        glob = top.enter_context(tc.tile_pool(name="glob", bufs=1))
        qrot = glob.tile([128, NQ, S], fmm)      # q^T, rope'd, per head
        krot = glob.tile([128, NKVL, S], fmm)    # k^T, rope'd, per kv head
        vaug = glob.tile([128, NKVL, ST, 132], fmm)  # v chunks + ones col @128
        mask_sb = glob.tile([128, 128], fmm)

        nc.gpsimd.dma_start(out=mask_sb, in_=mask_d[:, :])
        nc.vector.memset(vaug[:, :, :, 128:129], 1.0)

        # HAM warm-up: ~10 throwaway matmuls on a zeroed scratch tile keep
        # the PE busy across the 3.4us activity window while the first
        # input DMAs land, so real matmuls start at the full clock.
        with ExitStack() as wctx:
            wpool = wctx.enter_context(tc.tile_pool(name="warm", bufs=1))
            wps_pool = wctx.enter_context(
                tc.tile_pool(name="warmps", bufs=1, space="PSUM")
            )
            warm = wpool.tile([128, 512], fmm)
            nc.vector.memset(warm, 0.0)
            wps = wps_pool.tile([128, 512], f32)
            for _ in range(14):
                nc.tensor.matmul(wps, warm[:, 0:128], warm, start=True, stop=True)

        # ---------------- phase 1: projections + rope ----------------
        with ExitStack() as ph1:
            consts = ph1.enter_context(tc.tile_pool(name="p1const", bufs=1))
            hsp = ph1.enter_context(tc.tile_pool(name="p1hs", bufs=3))
            ropep = ph1.enter_context(tc.tile_pool(name="p1rope", bufs=3))
            qk_ps = ph1.enter_context(tc.tile_pool(name="p1qkps", bufs=6, space="PSUM"))
            v_ps = ph1.enter_context(tc.tile_pool(name="p1vps", bufs=2, space="PSUM"))

            def hs_load(nb, splits=(4, 8, 12, 16)):
                # few DMAs per 1 MiB block with 2+ KiB contiguous
                # per-partition lines (vs 512 B per-chunk lines --
                # descriptor count gates the kernel lead-in, not bandwidth)
                t = hsp.tile([128, HC, BW], fmm, name=f"hs_{nb}", tag="hs")
                lo = 0
                for hi in splits:
                    nc.sync.dma_start(
                        out=t[:, lo:hi, :], in_=hsT_d[nb, :, lo:hi, :]
                    )
                    lo = hi
                return t

            wq_sb = consts.tile([128, HC, 128 * NQ], fmm)
            wk_sb = consts.tile([128, HC, 128 * NKVL], fmm)
            wv_sb = consts.tile([128, HC, 128 * NKVL], fmm)
            cos_sb = consts.tile([128, S], fmm)
            sin_sb = consts.tile([128, S], fmm)
            # The scalar ring's first trigger sits behind the runtime's
            # ACT_TABLE_LOAD (up to ~12us in), so everything the first
            # matmuls need rides the sync ring in exact consumption order
            # for the nb=0 two-pass projection; the scalar ring only
            # carries the trig tables (first needed by the nb=0 rope,
            # ~10us after the first matmul).
            hs0 = hsp.tile([128, HC, BW], fmm, name="hs_0", tag="hs")
            nc.sync.dma_start(out=wq_sb[:, 0:1, :], in_=wq_d[:, 0:1, :])
            nc.sync.dma_start(out=hs0[:, 0:2, :], in_=hsT_d[0, :, 0:2, :])
            nc.sync.dma_start(out=wq_sb[:, 1:4, :], in_=wq_d[:, 1:4, :])
            nc.sync.dma_start(out=hs0[:, 2:4, :], in_=hsT_d[0, :, 2:4, :])
            nc.sync.dma_start(out=wq_sb[:, 4:8, :], in_=wq_d[:, 4:8, :])
            nc.sync.dma_start(out=hs0[:, 4:8, :], in_=hsT_d[0, :, 4:8, :])
            nc.sync.dma_start(out=wk_sb[:, 0:8, :], in_=wk_d[:, 0:8, :])
            nc.sync.dma_start(out=wq_sb[:, 8:12, :], in_=wq_d[:, 8:12, :])
            nc.sync.dma_start(out=wq_sb[:, 12:16, :], in_=wq_d[:, 12:16, :])
            nc.sync.dma_start(out=hs0[:, 8:12, :], in_=hsT_d[0, :, 8:12, :])
            nc.sync.dma_start(out=hs0[:, 12:16, :], in_=hsT_d[0, :, 12:16, :])
            nc.sync.dma_start(out=wk_sb[:, 8:16, :], in_=wk_d[:, 8:16, :])
            nc.sync.dma_start(out=wv_sb, in_=wv_d[:, :, :])
            hs_next = hs0

            nc.scalar.dma_start(out=cos_sb[:, 0:512], in_=cos_d[:, 0:512])
            nc.scalar.dma_start(out=sin_sb[:, 0:512], in_=sin_d[:, 0:512])
            nc.scalar.dma_start(out=cos_sb[:, 512:S], in_=cos_d[:, 512:S])
            nc.scalar.dma_start(out=sin_sb[:, 512:S], in_=sin_d[:, 512:S])

            def emit_rope(ps, mt, n0):
                if mt < NQ:
                    dest = qrot[:, mt, n0 : n0 + BW]
                else:
                    dest = krot[:, mt - NQ, n0 : n0 + BW]
                # rope: dest = ps * cos + swap_halves(ps) * (+/-)sin
                t_t = ropep.tile([128, BW], f32, tag="ropet")
                u_t = ropep.tile([128, BW], f32, tag="ropeu")
                nc.vector.tensor_mul(t_t, ps, cos_sb[:, n0 : n0 + BW])
                nc.vector.tensor_mul(
                    u_t[0:64, :], ps[64:128, :], sin_sb[0:64, n0 : n0 + BW]
                )
                nc.vector.tensor_mul(
                    u_t[64:128, :], ps[0:64, :], sin_sb[64:128, n0 : n0 + BW]
                )
                nc.vector.tensor_add(dest, t_t, u_t)

            def w_of(mt):
                if mt < NQ:
                    return wq_sb, mt
                return wk_sb, mt - NQ

            for nb in range(NB):
                n0 = nb * BW
                hs_t = hs_next
                if nb + 1 < NB:
                    hs_next = hs_load(nb + 1)

                if nb == 0:
                    # two half-contraction passes: pass A (chunks 0..7)
                    # only needs the lower half of hs/wq/wk, so the PE
                    # starts ~8us before the full weights have landed.
                    pss = []
                    for mt in range(NQ + NKVL):
                        ps = qk_ps.tile([128, BW], f32, name=f"qk0_{mt}", tag="ps")
                        pss.append(ps)
                    for half in (0, 1):
                        for mt in range(NQ + NKVL):
                            w_sb, mo = w_of(mt)
                            for c in range(8 * half, 8 * half + 8):
                                nc.tensor.matmul(
                                    pss[mt],
                                    w_sb[:, c, 128 * mo : 128 * mo + 128],
                                    hs_t[:, c, :],
                                    start=(c == 0),
                                    stop=(c == HC - 1),
                                )
                    for mt in range(NQ + NKVL):
                        emit_rope(pss[mt], mt, n0)
                else:
                    # q/k projections (transposed out) + rope
                    for mt in range(NQ + NKVL):
                        ps = qk_ps.tile([128, BW], f32)
                        w_sb, mo = w_of(mt)
                        for c in range(HC):
                            nc.tensor.matmul(
                                ps,
                                w_sb[:, c, 128 * mo : 128 * mo + 128],
                                hs_t[:, c, :],
                                start=(c == 0),
                                stop=(c == HC - 1),
                            )
                        emit_rope(ps, mt, n0)

                # v projection (NOT transposed): out[s, d_local]
                for st2 in range(BW // 128):
                    st = (BW // 128) * nb + st2
                    ps = v_ps.tile([128, 128 * NKVL], f32)
                    for c in range(HC):
                        nc.tensor.matmul(
                            ps,
                            hs_t[:, c, 128 * st2 : 128 * st2 + 128],
                            wv_sb[:, c, :],
                            start=(c == 0),
                            stop=(c == HC - 1),
                        )
                    for kv in range(NKVL):
                        # ScalarE copy: ACT is idle during phase 1, DVE is not
                        nc.scalar.copy(
                            vaug[:, kv, st, 0:128], ps[:, 128 * kv : 128 * kv + 128]
                        )

        # ---------------- phases 2+3 ----------------
        late = top.enter_context(tc.tile_pool(name="late", bufs=1))
        attnT = late.tile([128, NQ, S], fmm)     # attention out, transposed
        wo_sb = late.tile([128, NQ, H], fmm)
        nc.gpsimd.dma_start(out=wo_sb, in_=wo_d[:, :, :])

        # ---------------- phase 2: attention ----------------
        with ExitStack() as ph2:
            ptp = ph2.enter_context(tc.tile_pool(name="p2pt", bufs=2))
            s_ps = ph2.enter_context(tc.tile_pool(name="p2sps", bufs=2, space="PSUM"))
            pv_ps = ph2.enter_context(tc.tile_pool(name="p2pvps", bufs=2, space="PSUM"))
            stg = ph2.enter_context(tc.tile_pool(name="p2stg", bufs=4))
            smal = ph2.enter_context(tc.tile_pool(name="p2small", bufs=4))

            # PV(t) of a head only needs score stripes c <= t, so it is
            # interleaved into the head's own scores loop with a LAG-stripe
            # slack for the exp stream: the PE fills its exp-pacing bubbles
            # with PV work instead of idling.  The normalize chain
            # (DVE reciprocal/scale + DMA XBAR transpose) never touches
            # the PE, so it is emitted immediately after each PV group.
            # Across heads, the tail flush (the last LAG PV groups of head
            # a) interleaves with the first LAG score stripes of head a+1
            # so the ScalarE exp stream never pauses at head boundaries.
            LAG = 4
            pTs = {}

            def emit_pv(a, t):
                kv = a // 2
                pT = pTs[a]
                po = pv_ps.tile([128, 132], f32, name=f"pv_{a}_{t}", tag="pvps")
                for cc in range(t + 1):
                    lhsT = pT[
                        :,
                        STRIPE_OFF[cc] + 128 * (t - cc) : STRIPE_OFF[cc]
                        + 128 * (t - cc)
                        + 128,
                    ]
                    nc.tensor.matmul(
                        po[:, 0:129],
                        lhsT,
                        vaug[:, kv, cc, 0:129],
                        start=(cc == 0),
                        stop=(cc == t),
                    )
                r = smal.tile([128, 1], f32, name=f"r_{a}_{t}", tag="recip")
                nc.vector.reciprocal(r, po[:, 128:129])
                stage = stg.tile([128, 128], fmm, name=f"st_{a}_{t}", tag="stage")
                nc.vector.tensor_scalar_mul(stage, po[:, 0:128], r)
                nc.sync.dma_start(
                    out=attnT[:, a, 128 * t : 128 * t + 128],
                    in_=stage,
                    transpose=True,
                )

            def emit_stripe(a, c):
                # scores^T + exp for head a, stripe c (q >= 128c only).
                # psum tiles are 1024 wide (2 banks): 2 matmuls fill, one
                # wide ScalarE exp drains (fewer, longer activations
                # amortize ACT's per-instruction cost).
                kv = a // 2
                pT = pTs[a]
                off = STRIPE_OFF[c]
                qlen = STRIPE_LEN[c]
                lhsT = krot[:, kv, 128 * c : 128 * c + 128]
                for sb in range((qlen + 1535) // 1536):
                    q0 = 128 * c + 1536 * sb
                    w = min(1536, S - q0)
                    ps = s_ps.tile([128, 1536], f32, tag="sps")
                    for h in range(0, w, 512):
                        hw = min(512, w - h)
                        nc.tensor.matmul(
                            ps[:, h : h + hw],
                            lhsT,
                            qrot[:, a, q0 + h : q0 + h + hw],
                            start=True,
                            stop=True,
                        )
                    nc.scalar.activation(
                        pT[:, off + q0 - 128 * c : off + q0 - 128 * c + w],
                        ps[:, :w],
                        mybir.ActivationFunctionType.Exp,
                        scale=float(SCALE),
                    )
                # causal mask on the diagonal 128-block of this stripe
                nc.vector.tensor_mul(
                    pT[:, off : off + 128], pT[:, off : off + 128], mask_sb
                )

            for a in range(NQ):
                pTs[a] = ptp.tile([128, PT_TOTAL], fmm, name=f"pT_{a}", tag="pT")
                for c in range(ST):
                    emit_stripe(a, c)
                    if c >= LAG:
                        emit_pv(a, c - LAG)
                    elif a >= 1:
                        emit_pv(a - 1, ST - LAG + c)
            for t in range(ST - LAG, ST):
                emit_pv(NQ - 1, t)

        # ---------------- phase 3: o_proj partial ----------------
        with ExitStack() as ph3:
            o_ps = ph3.enter_context(tc.tile_pool(name="p3ops", bufs=6, space="PSUM"))
            ostg = ph3.enter_context(tc.tile_pool(name="p3stg", bufs=3))

            for mt in range(H // 128):
                orow = ostg.tile([128, S], fmm, tag="ostg")
                for ns in range(S // 512):
                    ps = o_ps.tile([128, 512], f32, tag="ops")
                    for a in range(NQ):
                        nc.tensor.matmul(
                            ps,
                            wo_sb[:, a, 128 * mt : 128 * mt + 128],
                            attnT[:, a, 512 * ns : 512 * ns + 512],
                            start=(a == 0),
                            stop=(a == NQ - 1),
                        )
                    # alternate the psum drain between DVE and ACT so
                    # neither engine gates the o_proj matmul stream
                    if ns % 2 == 0:
                        nc.vector.tensor_copy(orow[:, 512 * ns : 512 * (ns + 1)], ps)
                    else:
                        nc.scalar.copy(orow[:, 512 * ns : 512 * (ns + 1)], ps)
                nc.sync.dma_start(
                    out=outT_d[128 * mt : 128 * mt + 128, :], in_=orow
                )

    nc.finalize()
    return nc


def _rope_tables():
    inv_freq = 1.0 / (10000.0 ** (np.arange(0, D, 2, dtype=np.float32) / D))
    t = np.arange(S, dtype=np.float32)[:, None]
    freqs = t * inv_freq[None, :]          # [S, 64]
    cos = np.cos(freqs).astype(np.float32)  # [S, 64]
    sin = np.sin(freqs).astype(np.float32)
    mdt = np.dtype(MM_DT)
    cosf = np.concatenate([cos, cos], axis=1).T.astype(mdt)    # [128, S]
    sins = np.concatenate([-sin, sin], axis=1).T.astype(mdt)   # [128, S]
    return np.ascontiguousarray(cosf), np.ascontiguousarray(sins)


def _prep_in_maps(hidden_states, Wq, Wk, Wv, Wo):
    mdt = np.dtype(MM_DT)
    cosf, sins = _rope_tables()
    mask = np.triu(np.ones((128, 128), dtype=mdt))  # [j, q]: 1 if j <= q

    hsT_blocks = []
    for b in range(B):
        hsT = hidden_states[b].T  # [H, S]
        blk = np.ascontiguousarray(
            hsT.reshape(HC, 128, NB, BW).transpose(2, 1, 0, 3).astype(mdt)
        )  # [NB, 128, HC, BW]
        hsT_blocks.append(blk)

    in_maps = []
    for i in range(8):
        b, g = i // 4, i % 4
        wq = np.ascontiguousarray(
            Wq[512 * g : 512 * (g + 1), :].reshape(512, HC, 128).transpose(2, 1, 0).astype(mdt)
        )
        wk = np.ascontiguousarray(
            Wk[256 * g : 256 * (g + 1), :].reshape(256, HC, 128).transpose(2, 1, 0).astype(mdt)
        )
        wv = np.ascontiguousarray(
            Wv[256 * g : 256 * (g + 1), :].reshape(256, HC, 128).transpose(2, 1, 0).astype(mdt)
        )
        wo = np.ascontiguousarray(
            Wo[:, 512 * g : 512 * (g + 1)].reshape(H, NQ, 128).transpose(2, 1, 0).astype(mdt)
        )
        in_maps.append(
            {
                "hsT": hsT_blocks[b],
                "wq": wq,
                "wk": wk,
                "wv": wv,
                "wo": wo,
                "cosf": cosf,
                "sins": sins,
                "mask": mask,
            }
        )
    return in_maps


def _run(in_maps, **kwargs):
    from concourse.bass_utils import run_bass_kernel_spmd

    if "prog" not in _CACHE:
        _CACHE["prog"] = _build_program()
    nc = _CACHE["prog"]
    return run_bass_kernel_spmd(nc, in_maps, core_ids=list(range(8)), **kwargs)


def _gather(results):
    out = np.empty((B, S, H), dtype=np.float32)
    for b in range(B):
        acc = results[4 * b + 0]["outT"].astype(np.float32)
        for g in range(1, 4):
            acc += results[4 * b + g]["outT"].astype(np.float32)
        out[b] = acc.T
    return out


def kernel(hidden_states, Wq, Wk, Wv, Wo):
    hidden_states = np.asarray(hidden_states, dtype=np.float32)
    Wq = np.asarray(Wq, dtype=np.float32)
    Wk = np.asarray(Wk, dtype=np.float32)
    Wv = np.asarray(Wv, dtype=np.float32)
    Wo = np.asarray(Wo, dtype=np.float32)
    in_maps = _prep_in_maps(hidden_states, Wq, Wk, Wv, Wo)
    res = _run(in_maps)
    return _gather(res.results)


# revision 56
# speedup vs baseline: 1.0788x; 1.0788x over previous
"""Trainium2 Bass kernel for GQA attention block (nn_Attention_20272245637793).

Reference computation (B=2, S=2048, H=2048, 16 q heads / 8 kv heads, D=128):
    q = hs @ Wq.T ; k = hs @ Wk.T ; v = hs @ Wv.T
    rope(q), rope(k); causal softmax(q k^T / sqrt(D)) @ v ; out @ Wo.T

Sharding (8 cores): core i = (b, g) with b = i // 4 (data-parallel over
batch), g = i % 4 (tensor-parallel over kv-head groups; kv heads {2g, 2g+1},
q heads {4g..4g+3}).  Each core computes 1/8 of every GEMM and a partial
o_proj over its 512 head-dims; the host sums the 4 partials per batch
(cheap, off-device) instead of an on-device all-reduce.

Per-core dataflow (matmul operands fp16, PSUM accumulation fp32):
  warm-up: ~18 throwaway matmuls on a zeroed tile keep the PE busy across
           the HAM 3.4us activity window while the first DMAs land, so
           real matmuls start at the full (unthrottled) clock.
  phase 1: QK^T projections produce q^T/k^T in [d_head(part) x S(free)]
           layout directly (weights stationary, hs^T moving); RoPE applied
           on the PSUM->SBUF path with 4 DVE ops per tile using
           host-precomputed cos / (+/-)sin tables.  The first hs block
           runs its contraction in two half passes (chunks 0..7 then
           8..15) so the PE starts before the full weights have landed.
           V is computed NON-transposed ([S x d]) with hs^T slices as the
           stationary operand, ones-column appended (denominator trick),
           drained by the otherwise-idle ScalarE.
  phase 2: per q head: scores^T tiles = K^T-chunk (stationary) @ q^T
           (moving) -> PSUM [k_pos(part) x q(free)] in 1024-wide 2-bank
           tiles (2 matmuls fill, ONE wide exp drains via ScalarE with
           scale=1/sqrt(D) fused); causal handled by skipping fully-masked
           tiles + one 0/1 mask multiply on diagonal tiles.  PV matmuls
           (exp'd score tiles stationary against V'; output column 128 is
           the softmax denominator) interleave into the scores loop with a
           4-stripe lag, filling the PE bubbles left by the pacing exp
           stream.  Normalize = DVE reciprocal + per-partition scalar
           multiply into an SBUF staging tile; the [q x d] -> [d x q]
           transpose for o_proj runs OFF the PE as a DMA XBAR transpose
           on the idle sync ring.
  phase 3: o_proj partial out^T[h, s] = Wo-slice^T (stationary) @ attn^T
           (moving), mt-outer so each 128-row strip is stored with ONE
           wide fp16 DMA; psum drains alternate DVE/ScalarE; host sums/
           transposes partials in fp32.

DMA layout: all input tensors are shaped so per-partition lines are
2-16 KiB contiguous, split and ring-ordered by consumption order --
descriptor count and trigger serialization gate the kernel lead-in,
not bandwidth.

Built on bacc.Bacc (not raw bass.Bass): TRN2 instructions can carry at most
ONE semaphore wait; Bacc.compile() legalizes multi-wait instructions via
move_matmul_waits_to_ldweights + generate_event_semaphores.
"""

import sys

sys.path.insert(0, "/opt/trn_rl_repo")

import numpy as np
from contextlib import ExitStack

B = 2
S = 2048
H = 2048
D = 128
NQ = 4          # q heads per core
NKVL = 2        # kv heads per core
HC = H // 128   # 16 h-chunks (contraction)
NB = 8          # hs^T column blocks of 256 for projections
BW = S // NB    # 256
ST = S // 128   # 16 s-tiles / k-chunks / q-tiles
SCALE = 1.0 / np.sqrt(D)

# stripe c of the exp'd transposed scores covers q in [128c, S); offsets of
# the stripes packed into one [128, sum] sbuf tile
STRIPE_LEN = [S - 128 * c for c in range(ST)]
STRIPE_OFF = np.concatenate([[0], np.cumsum(STRIPE_LEN)]).tolist()
PT_TOTAL = STRIPE_OFF[-1]  # 17408

# dtype for matmul operands (PSUM accumulation is always fp32).  fp32 runs
# every matmul as a hi/lo double pass on the PE; float16 is single-pass
# (2x) with ~1e-3 accuracy, and halves DMA/SBUF for those tensors.
MM_DT = "float16"

_CACHE = {}


def _build_program():
    import concourse.tile as tile
    from concourse import bacc, mybir

    f32 = mybir.dt.float32
    fmm = getattr(mybir.dt, MM_DT)
    nc = bacc.Bacc()

    hsT_d = nc.declare_dram_parameter("hsT", [NB, 128, HC, BW], fmm, isOutput=False)
    wq_d = nc.declare_dram_parameter("wq", [128, HC, 128 * NQ], fmm, isOutput=False)
    wk_d = nc.declare_dram_parameter("wk", [128, HC, 128 * NKVL], fmm, isOutput=False)
    wv_d = nc.declare_dram_parameter("wv", [128, HC, 128 * NKVL], fmm, isOutput=False)
    wo_d = nc.declare_dram_parameter("wo", [128, NQ, H], fmm, isOutput=False)
    cos_d = nc.declare_dram_parameter("cosf", [128, S], fmm, isOutput=False)
    sin_d = nc.declare_dram_parameter("sins", [128, S], fmm, isOutput=False)
    mask_d = nc.declare_dram_parameter("mask", [128, 128], fmm, isOutput=False)
    outT_d = nc.declare_dram_parameter("outT", [H, S], fmm, isOutput=True)

    with tile.TileContext(nc) as tc, ExitStack() as top:
        # tiles that live across phases
        glob = top.enter_context(tc.tile_pool(name="glob", bufs=1))
        qrot = glob.tile([128, NQ, S], fmm)      # q^T, rope'd, per head
        krot = glob.tile([128, NKVL, S], fmm)    # k^T, rope'd, per kv head
        vaug = glob.tile([128, NKVL, ST, 132], fmm)  # v chunks + ones col @128
        mask_sb = glob.tile([128, 128], fmm)

        nc.gpsimd.dma_start(out=mask_sb, in_=mask_d[:, :])
        nc.vector.memset(vaug[:, :, :, 128:129], 1.0)

        # HAM warm-up: ~10 throwaway matmuls on a zeroed scratch tile keep
        # the PE busy across the 3.4us activity window while the first
        # input DMAs land, so real matmuls start at the full clock.
        with ExitStack() as wctx:
            wpool = wctx.enter_context(tc.tile_pool(name="warm", bufs=1))
            wps_pool = wctx.enter_context(
                tc.tile_pool(name="warmps", bufs=1, space="PSUM")
            )
            warm = wpool.tile([128, 512], fmm)
            nc.vector.memset(warm, 0.0)
            wps = wps_pool.tile([128, 512], f32)
            for _ in range(10):
                nc.tensor.matmul(wps, warm[:, 0:128], warm, start=True, stop=True)

        # ---------------- phase 1: projections + rope ----------------
        with ExitStack() as ph1:
            consts = ph1.enter_context(tc.tile_pool(name="p1const", bufs=1))
            hsp = ph1.enter_context(tc.tile_pool(name="p1hs", bufs=3))
            ropep = ph1.enter_context(tc.tile_pool(name="p1rope", bufs=3))
            qk_ps = ph1.enter_context(tc.tile_pool(name="p1qkps", bufs=6, space="PSUM"))
            v_ps = ph1.enter_context(tc.tile_pool(name="p1vps", bufs=2, space="PSUM"))

            def hs_load(nb, splits=(4, 8, 12, 16)):
                # few DMAs per 1 MiB block with 2+ KiB contiguous
                # per-partition lines (vs 512 B per-chunk lines --
                # descriptor count gates the kernel lead-in, not bandwidth)
                t = hsp.tile([128, HC, BW], fmm, name=f"hs_{nb}", tag="hs")
                lo = 0
                for hi in splits:
                    nc.sync.dma_start(
                        out=t[:, lo:hi, :], in_=hsT_d[nb, :, lo:hi, :]
                    )
                    lo = hi
                return t

            wq_sb = consts.tile([128, HC, 128 * NQ], fmm)
            wk_sb = consts.tile([128, HC, 128 * NKVL], fmm)
            wv_sb = consts.tile([128, HC, 128 * NKVL], fmm)
            cos_sb = consts.tile([128, S], fmm)
            sin_sb = consts.tile([128, S], fmm)
            # The scalar ring's first trigger sits behind the runtime's
            # ACT_TABLE_LOAD (up to ~12us in), so everything the first
            # matmuls need rides the sync ring in exact consumption order
            # for the nb=0 two-pass projection; the scalar ring only
            # carries the trig tables (first needed by the nb=0 rope,
            # ~10us after the first matmul).
            hs0 = hsp.tile([128, HC, BW], fmm, name="hs_0", tag="hs")
            nc.sync.dma_start(out=wq_sb[:, 0:1, :], in_=wq_d[:, 0:1, :])
            nc.sync.dma_start(out=hs0[:, 0:2, :], in_=hsT_d[0, :, 0:2, :])
            nc.sync.dma_start(out=wq_sb[:, 1:4, :], in_=wq_d[:, 1:4, :])
            nc.sync.dma_start(out=hs0[:, 2:4, :], in_=hsT_d[0, :, 2:4, :])
            nc.sync.dma_start(out=wq_sb[:, 4:8, :], in_=wq_d[:, 4:8, :])
            nc.sync.dma_start(out=hs0[:, 4:8, :], in_=hsT_d[0, :, 4:8, :])
            nc.sync.dma_start(out=wk_sb[:, 0:8, :], in_=wk_d[:, 0:8, :])
            nc.sync.dma_start(out=wq_sb[:, 8:12, :], in_=wq_d[:, 8:12, :])
            nc.sync.dma_start(out=wq_sb[:, 12:16, :], in_=wq_d[:, 12:16, :])
            nc.sync.dma_start(out=hs0[:, 8:12, :], in_=hsT_d[0, :, 8:12, :])
            nc.sync.dma_start(out=hs0[:, 12:16, :], in_=hsT_d[0, :, 12:16, :])
            nc.sync.dma_start(out=wk_sb[:, 8:16, :], in_=wk_d[:, 8:16, :])
            nc.sync.dma_start(out=wv_sb, in_=wv_d[:, :, :])
            hs_next = hs0

            nc.scalar.dma_start(out=cos_sb[:, 0:512], in_=cos_d[:, 0:512])
            nc.scalar.dma_start(out=sin_sb[:, 0:512], in_=sin_d[:, 0:512])
            nc.scalar.dma_start(out=cos_sb[:, 512:S], in_=cos_d[:, 512:S])
            nc.scalar.dma_start(out=sin_sb[:, 512:S], in_=sin_d[:, 512:S])

            def emit_rope(ps, mt, n0):
                if mt < NQ:
                    dest = qrot[:, mt, n0 : n0 + BW]
                else:
                    dest = krot[:, mt - NQ, n0 : n0 + BW]
                # rope: dest = ps * cos + swap_halves(ps) * (+/-)sin
                t_t = ropep.tile([128, BW], f32, tag="ropet")
                u_t = ropep.tile([128, BW], f32, tag="ropeu")
                nc.vector.tensor_mul(t_t, ps, cos_sb[:, n0 : n0 + BW])
                nc.vector.tensor_mul(
                    u_t[0:64, :], ps[64:128, :], sin_sb[0:64, n0 : n0 + BW]
                )
                nc.vector.tensor_mul(
                    u_t[64:128, :], ps[0:64, :], sin_sb[64:128, n0 : n0 + BW]
                )
                nc.vector.tensor_add(dest, t_t, u_t)

            def w_of(mt):
                if mt < NQ:
                    return wq_sb, mt
                return wk_sb, mt - NQ

            for nb in range(NB):
                n0 = nb * BW
                hs_t = hs_next
                if nb + 1 < NB:
                    hs_next = hs_load(nb + 1)

                if nb == 0:
                    # two half-contraction passes: pass A (chunks 0..7)
                    # only needs the lower half of hs/wq/wk, so the PE
                    # starts ~8us before the full weights have landed.
                    pss = []
                    for mt in range(NQ + NKVL):
                        ps = qk_ps.tile([128, BW], f32, name=f"qk0_{mt}", tag="ps")
                        pss.append(ps)
                    for half in (0, 1):
                        for mt in range(NQ + NKVL):
                            w_sb, mo = w_of(mt)
                            for c in range(8 * half, 8 * half + 8):
                                nc.tensor.matmul(
                                    pss[mt],
                                    w_sb[:, c, 128 * mo : 128 * mo + 128],
                                    hs_t[:, c, :],
                                    start=(c == 0),
                                    stop=(c == HC - 1),
                                )
                    for mt in range(NQ + NKVL):
                        emit_rope(pss[mt], mt, n0)
                else:
                    # q/k projections (transposed out) + rope
                    for mt in range(NQ + NKVL):
                        ps = qk_ps.tile([128, BW], f32)
                        w_sb, mo = w_of(mt)
                        for c in range(HC):
                            nc.tensor.matmul(
                                ps,
                                w_sb[:, c, 128 * mo : 128 * mo + 128],
                                hs_t[:, c, :],
                                start=(c == 0),
                                stop=(c == HC - 1),
                            )
                        emit_rope(ps, mt, n0)

                # v projection (NOT transposed): out[s, d_local]
                for st2 in range(BW // 128):
                    st = (BW // 128) * nb + st2
                    ps = v_ps.tile([128, 128 * NKVL], f32)
                    for c in range(HC):
                        nc.tensor.matmul(
                            ps,
                            hs_t[:, c, 128 * st2 : 128 * st2 + 128],
                            wv_sb[:, c, :],
                            start=(c == 0),
                            stop=(c == HC - 1),
                        )
                    for kv in range(NKVL):
                        # ScalarE copy: ACT is idle during phase 1, DVE is not
                        nc.scalar.copy(
                            vaug[:, kv, st, 0:128], ps[:, 128 * kv : 128 * kv + 128]
                        )

        # ---------------- phases 2+3 ----------------
        late = top.enter_context(tc.tile_pool(name="late", bufs=1))
        attnT = late.tile([128, NQ, S], fmm)     # attention out, transposed
        wo_sb = late.tile([128, NQ, H], fmm)
        # heads-0/1 o_proj partial for the first two 512-col s-blocks,
        # computed during heads-2/3's attention (it only needs attnT[0..1])
        # and combined with the heads-2/3 psum in phase 3
        o01 = late.tile([128, H // 128, 1024], fmm)
        nc.gpsimd.dma_start(out=wo_sb, in_=wo_d[:, :, :])

        # ---------------- phase 2: attention ----------------
        with ExitStack() as ph2:
            ptp = ph2.enter_context(tc.tile_pool(name="p2pt", bufs=2))
            s_ps = ph2.enter_context(tc.tile_pool(name="p2sps", bufs=3, space="PSUM"))
            pv_ps = ph2.enter_context(tc.tile_pool(name="p2pvps", bufs=2, space="PSUM"))
            stg = ph2.enter_context(tc.tile_pool(name="p2stg", bufs=4))
            smal = ph2.enter_context(tc.tile_pool(name="p2small", bufs=4))

            # PV(t) of a head only needs score stripes c <= t, so it is
            # interleaved into the head's own scores loop with a LAG-stripe
            # slack for the exp stream: the PE fills its exp-pacing bubbles
            # with PV work instead of idling.  The normalize chain
            # (DVE reciprocal/scale + DMA XBAR transpose) never touches
            # the PE, so it is emitted immediately after each PV group.
            # Across heads, the tail flush (the last LAG PV groups of head
            # a) interleaves with the first LAG score stripes of head a+1
            # so the ScalarE exp stream never pauses at head boundaries.
            LAG = 4
            pTs = {}

            # heads-0/1 o_proj groups for s-blocks 0/1, interleaved into
            # heads 2/3 to fill the PE bubbles left by the exp pacing.
            # Their psum shares the pvps slots (same one-bank size class).
            o01_groups = [(mt, ns) for ns in (0, 1) for mt in range(H // 128)]
            o01_idx = [0]

            def emit_o01():
                if o01_idx[0] >= len(o01_groups):
                    return
                mt, ns = o01_groups[o01_idx[0]]
                o01_idx[0] += 1
                ps = pv_ps.tile([128, 512], f32, name=f"o01_{mt}_{ns}", tag="pvps")
                for a01 in (0, 1):
                    nc.tensor.matmul(
                        ps,
                        wo_sb[:, a01, 128 * mt : 128 * mt + 128],
                        attnT[:, a01, 512 * ns : 512 * ns + 512],
                        start=(a01 == 0),
                        stop=(a01 == 1),
                    )
                nc.vector.tensor_copy(o01[:, mt, 512 * ns : 512 * (ns + 1)], ps)

            def emit_pv(a, t):
                kv = a // 2
                pT = pTs[a]
                po = pv_ps.tile([128, 132], f32, name=f"pv_{a}_{t}", tag="pvps")
                for cc in range(t + 1):
                    lhsT = pT[
                        :,
                        STRIPE_OFF[cc] + 128 * (t - cc) : STRIPE_OFF[cc]
                        + 128 * (t - cc)
                        + 128,
                    ]
                    nc.tensor.matmul(
                        po[:, 0:129],
                        lhsT,
                        vaug[:, kv, cc, 0:129],
                        start=(cc == 0),
                        stop=(cc == t),
                    )
                r = smal.tile([128, 1], f32, name=f"r_{a}_{t}", tag="recip")
                nc.vector.reciprocal(r, po[:, 128:129])
                stage = stg.tile([128, 128], fmm, name=f"st_{a}_{t}", tag="stage")
                nc.vector.tensor_scalar_mul(stage, po[:, 0:128], r)
                nc.sync.dma_start(
                    out=attnT[:, a, 128 * t : 128 * t + 128],
                    in_=stage,
                    transpose=True,
                )

            def emit_stripe(a, c):
                # scores^T + exp for head a, stripe c (q >= 128c only).
                # psum tiles are 1024 wide (2 banks): 2 matmuls fill, one
                # wide ScalarE exp drains (fewer, longer activations
                # amortize ACT's per-instruction cost).
                kv = a // 2
                pT = pTs[a]
                off = STRIPE_OFF[c]
                qlen = STRIPE_LEN[c]
                lhsT = krot[:, kv, 128 * c : 128 * c + 128]
                for sb in range((qlen + 1023) // 1024):
                    q0 = 128 * c + 1024 * sb
                    w = min(1024, S - q0)
                    ps = s_ps.tile([128, 1024], f32, tag="sps")
                    for h in range(0, w, 512):
                        hw = min(512, w - h)
                        nc.tensor.matmul(
                            ps[:, h : h + hw],
                            lhsT,
                            qrot[:, a, q0 + h : q0 + h + hw],
                            start=True,
                            stop=True,
                        )
                    nc.scalar.activation(
                        pT[:, off + q0 - 128 * c : off + q0 - 128 * c + w],
                        ps[:, :w],
                        mybir.ActivationFunctionType.Exp,
                        scale=float(SCALE),
                    )
                # causal mask on the diagonal 128-block of this stripe
                nc.vector.tensor_mul(
                    pT[:, off : off + 128], pT[:, off : off + 128], mask_sb
                )

            for a in range(NQ):
                pTs[a] = ptp.tile([128, PT_TOTAL], fmm, name=f"pT_{a}", tag="pT")
                for c in range(ST):
                    emit_stripe(a, c)
                    if c >= LAG:
                        emit_pv(a, c - LAG)
                    elif a >= 1:
                        emit_pv(a - 1, ST - LAG + c)
                    # one group per iteration roughly matches the PE idle
                    # generated by the exp pacing; denser insertion would
                    # starve the exp stream instead of filling bubbles
                    if a > 2 or (a == 2 and c >= 5):
                        emit_o01()
            for t in range(ST - LAG, ST):
                emit_pv(NQ - 1, t)
                emit_o01()
            while o01_idx[0] < len(o01_groups):
                emit_o01()

        # ---------------- phase 3: o_proj partial ----------------
        with ExitStack() as ph3:
            o_ps = ph3.enter_context(tc.tile_pool(name="p3ops", bufs=6, space="PSUM"))
            ostg = ph3.enter_context(tc.tile_pool(name="p3stg", bufs=3))

            for mt in range(H // 128):
                orow = ostg.tile([128, S], fmm, tag="ostg")
                for ns in range(S // 512):
                    ps = o_ps.tile([128, 512], f32, tag="ops")
                    heads = (2, 3) if ns < 2 else range(NQ)
                    first, last = heads[0] if ns < 2 else 0, 3
                    for a in heads:
                        nc.tensor.matmul(
                            ps,
                            wo_sb[:, a, 128 * mt : 128 * mt + 128],
                            attnT[:, a, 512 * ns : 512 * ns + 512],
                            start=(a == first),
                            stop=(a == last),
                        )
                    if ns < 2:
                        # combine with the heads-0/1 partial staged in ph2
                        nc.vector.tensor_add(
                            orow[:, 512 * ns : 512 * (ns + 1)],
                            ps,
                            o01[:, mt, 512 * ns : 512 * (ns + 1)],
                        )
                    elif ns == 2:
                        nc.vector.tensor_copy(orow[:, 512 * ns : 512 * (ns + 1)], ps)
                    else:
                        nc.scalar.copy(orow[:, 512 * ns : 512 * (ns + 1)], ps)
                nc.sync.dma_start(
                    out=outT_d[128 * mt : 128 * mt + 128, :], in_=orow
                )

    nc.finalize()
    return nc


def _rope_tables():
    inv_freq = 1.0 / (10000.0 ** (np.arange(0, D, 2, dtype=np.float32) / D))
    t = np.arange(S, dtype=np.float32)[:, None]
    freqs = t * inv_freq[None, :]          # [S, 64]
    cos = np.cos(freqs).astype(np.float32)  # [S, 64]
    sin = np.sin(freqs).astype(np.float32)
    mdt = np.dtype(MM_DT)
    cosf = np.concatenate([cos, cos], axis=1).T.astype(mdt)    # [128, S]
    sins = np.concatenate([-sin, sin], axis=1).T.astype(mdt)   # [128, S]
    return np.ascontiguousarray(cosf), np.ascontiguousarray(sins)


def _prep_in_maps(hidden_states, Wq, Wk, Wv, Wo):
    mdt = np.dtype(MM_DT)
    cosf, sins = _rope_tables()
    mask = np.triu(np.ones((128, 128), dtype=mdt))  # [j, q]: 1 if j <= q

    hsT_blocks = []
    for b in range(B):
        hsT = hidden_states[b].T  # [H, S]
        blk = np.ascontiguousarray(
            hsT.reshape(HC, 128, NB, BW).transpose(2, 1, 0, 3).astype(mdt)
        )  # [NB, 128, HC, BW]
        hsT_blocks.append(blk)

    in_maps = []
    for i in range(8):
        b, g = i // 4, i % 4
        wq = np.ascontiguousarray(
            Wq[512 * g : 512 * (g + 1), :].reshape(512, HC, 128).transpose(2, 1, 0).astype(mdt)
        )
        wk = np.ascontiguousarray(
            Wk[256 * g : 256 * (g + 1), :].reshape(256, HC, 128).transpose(2, 1, 0).astype(mdt)
        )
        wv = np.ascontiguousarray(
            Wv[256 * g : 256 * (g + 1), :].reshape(256, HC, 128).transpose(2, 1, 0).astype(mdt)
        )
        wo = np.ascontiguousarray(
            Wo[:, 512 * g : 512 * (g + 1)].reshape(H, NQ, 128).transpose(2, 1, 0).astype(mdt)
        )
        in_maps.append(
            {
                "hsT": hsT_blocks[b],
                "wq": wq,
                "wk": wk,
                "wv": wv,
                "wo": wo,
                "cosf": cosf,
                "sins": sins,
                "mask": mask,
            }
        )
    return in_maps


def _run(in_maps, **kwargs):
    from concourse.bass_utils import run_bass_kernel_spmd

    if "prog" not in _CACHE:
        _CACHE["prog"] = _build_program()
    nc = _CACHE["prog"]
    return run_bass_kernel_spmd(nc, in_maps, core_ids=list(range(8)), **kwargs)


def _gather(results):
    out = np.empty((B, S, H), dtype=np.float32)
    for b in range(B):
        acc = results[4 * b + 0]["outT"].astype(np.float32)
        for g in range(1, 4):
            acc += results[4 * b + g]["outT"].astype(np.float32)
        out[b] = acc.T
    return out


def kernel(hidden_states, Wq, Wk, Wv, Wo):
    hidden_states = np.asarray(hidden_states, dtype=np.float32)
    Wq = np.asarray(Wq, dtype=np.float32)
    Wk = np.asarray(Wk, dtype=np.float32)
    Wv = np.asarray(Wv, dtype=np.float32)
    Wo = np.asarray(Wo, dtype=np.float32)
    in_maps = _prep_in_maps(hidden_states, Wq, Wk, Wv, Wo)
    res = _run(in_maps)
    return _gather(res.results)
